# revision 22
# baseline (speedup 1.0000x reference)
"""Trainium2 Bass kernel: 16-head RoPE attention (B=2, L=2048, HIDDEN=1024).

Sharding: 8 cores = 2 batches x 4 head-groups (4 heads per core).
Each core computes q/k/v projections for its 4 heads (feature-major),
RoPE, scores-transposed [k,q] per head, exp (no max subtraction --
scores are ~N(0,1)), PV with a ones-column in V to get softmax sums,
normalization, and a partial output projection [1024, 2048].
Host sums the 4 partials per batch and transposes back.

v2 schedule: chunked weight DMAs for a fast start, attention for
(pair 0, c=0) interleaved into the projection t-loop so the scalar
engine (exp) saturates early, and the softmax-sum broadcast done with
a stride-0 DMA instead of gpsimd PartitionBroadcast.
"""

import numpy as np
from contextlib import ExitStack

from concourse import bacc, tile, mybir
from concourse.bass import ts
from concourse.bass_utils import run_bass_kernel_spmd

HIDDEN = 1024
HEADS = 16
HD = 64
L = 2048
B = 2
BASE = 10000.0

P = 128
E_LOCAL = 256          # 4 heads per core
N_PAIRS = 2            # head pairs per core (2 heads on 128 partitions)
HC = HIDDEN // P       # 8 hidden chunks
TC = 512               # token chunk (matmul free dim)
N_TC = L // TC         # 4
N_TT = L // P          # 16 token tiles (for v / k-tiles)
SCALE = 1.0 / 8.0      # 1/sqrt(HD)

F32 = mybir.dt.float32
F32R = mybir.dt.float32r
BF16 = mybir.dt.bfloat16
AF = mybir.ActivationFunctionType
ALU = mybir.AluOpType


def r(ap):
    """View an fp32 AP as float32r for full-rate PE matmuls."""
    return ap.bitcast(F32R)


def build_program():
    nc = bacc.Bacc(None, target_bir_lowering=False)
    names = {}
    with tile.TileContext(nc) as tc:
        ctx = ExitStack()
        with ctx:
            dram = ctx.enter_context(tc.tile_pool(name="dram", bufs=1, space="DRAM"))
            xT_d = dram.tile([HIDDEN, L], F32, kind="ExternalInput", name="xT")
            wq_d = dram.tile([HIDDEN, E_LOCAL], F32, kind="ExternalInput", name="wq")
            wk_d = dram.tile([HIDDEN, E_LOCAL], F32, kind="ExternalInput", name="wk")
            wv_d = dram.tile([HIDDEN, E_LOCAL], F32, kind="ExternalInput", name="wv")
            wo_d = dram.tile([E_LOCAL, HIDDEN], F32, kind="ExternalInput", name="wo")
            cos_d = dram.tile([P, L], F32, kind="ExternalInput", name="cosT")
            sin_d = dram.tile([P, L], F32, kind="ExternalInput", name="sinT")
            out_d = dram.tile([HIDDEN, L], F32, kind="ExternalOutput", name="outT")
            names["in"] = [t.tensor.name for t in (xT_d, wq_d, wk_d, wv_d, wo_d, cos_d, sin_d)]
            names["out"] = out_d.tensor.name

            # ---------------- persistent SBUF ----------------
            const = ctx.enter_context(tc.tile_pool(name="const", bufs=1))
            HH = HC // 2
            wq_sbs = [const.tile([P, HH, E_LOCAL], F32R, name=f"wq{i}") for i in range(2)]
            wk_sbs = [const.tile([P, HH, E_LOCAL], F32R, name=f"wk{i}") for i in range(2)]
            wv_sbs = [const.tile([P, HH, E_LOCAL], F32R, name=f"wv{i}") for i in range(2)]
            wo_sb = const.tile([P, 2, HIDDEN], F32R)
            cos_sbs = [const.tile([P, L // 2], F32, name=f"cos{i}") for i in range(2)]
            sin_sbs = [const.tile([P, L // 2], F32, name=f"sin{i}") for i in range(2)]
            # head loads: the sync sequencer's direct DMAs occupy it
            # proportionally to bytes, so interleave half-tensors in the
            # exact order the first projection matmuls consume them
            wq_v = wq_d[:].rearrange("(c p) e -> p c e", p=P)
            wk_v = wk_d[:].rearrange("(c p) e -> p c e", p=P)
            wv_v = wv_d[:].rearrange("(c p) e -> p c e", p=P)
            # ---------------- pools ----------------
            xpool = ctx.enter_context(tc.tile_pool(name="xpool", bufs=2))
            rope_t = ctx.enter_context(tc.tile_pool(name="rope", bufs=2))
            expp = ctx.enter_context(tc.tile_pool(name="expp", bufs=5))
            nrm = ctx.enter_context(tc.tile_pool(name="nrm", bufs=1))
            outst = ctx.enter_context(tc.tile_pool(name="outst", bufs=4))
            oacc = ctx.enter_context(tc.tile_pool(name="oacc", bufs=8))
            x0ts = [oacc.tile([P, TC], F32R, name="obA") for _ in range(HC)]
            nc.sync.dma_start(wq_sbs[0][:], r(wq_v[:, 0:HH, :]))
            nc.scalar.dma_start(wk_sbs[0][:], r(wk_v[:, 0:HH, :]))
            for h in range(HH):
                nc.sync.dma_start(x0ts[h][:], r(xT_d[ts(h, P), ts(0, TC)]))
            nc.scalar.dma_start(wk_sbs[1][:], r(wk_v[:, HH:HC, :]))
            nc.sync.dma_start(wq_sbs[1][:], r(wq_v[:, HH:HC, :]))
            for h in range(HH, HC):
                nc.sync.dma_start(x0ts[h][:], r(xT_d[ts(h, P), ts(0, TC)]))
            nc.scalar.dma_start(sin_sbs[0][:], sin_d[:, 0 : L // 2])
            nc.sync.dma_start(cos_sbs[0][:], cos_d[:, 0 : L // 2])
            nc.scalar.dma_start(sin_sbs[1][:], sin_d[:, L // 2 :])
            nc.sync.dma_start(wv_sbs[0][:], r(wv_v[:, 0:HH, :]))
            nc.sync.dma_start(wv_sbs[1][:], r(wv_v[:, HH:HC, :]))
            nc.scalar.dma_start(wo_sb[:], r(wo_d[:].rearrange("(c p) f -> p c f", p=P)))
            xt1 = xpool.tile([P, HC, TC], F32R, name="xt")
            nc.sync.dma_start(
                xt1[:], r(xT_d[:, ts(1, TC)].rearrange("(c p) f -> p c f", p=P))
            )
            nc.sync.dma_start(cos_sbs[1][:], cos_d[:, L // 2 :])

            # rope'd q and k, feature-major: per pair [128, L]
            qkro = ctx.enter_context(tc.tile_pool(name="qkro", bufs=1))
            q_ro = [qkro.tile([P, L], F32R, name=f"q_ro{p}") for p in range(N_PAIRS)]
            k_ro = [qkro.tile([P, L], F32R, name=f"k_ro{p}") for p in range(N_PAIRS)]
            # v token-major with ones columns: [128 tok, tt, 4*65]
            v_all = qkro.tile([P, N_TT, 4 * (HD + 1)], BF16)
            v4 = v_all[:].rearrange("p t (g c) -> p t g c", g=4)
            ones_sb = qkro.tile([P, N_TT], BF16)
            nc.vector.memset(ones_sb[:], 1.0)
            for g in range(4):
                nc.vector.tensor_copy(
                    v_all[:, :, g * (HD + 1) + HD : g * (HD + 1) + HD + 1],
                    ones_sb[:].rearrange("p (a b) -> p a b", b=1),
                )
            # normalized attention output, feature-major per pair [128, L]
            o_sb = [qkro.tile([P, L], F32R, name=f"o_sb{p}") for p in range(N_PAIRS)]


            def rope_chunk(dst, ps_tile, t, shuf_eng, t2_gps=False):
                """psum [128, TC] -> dst[:, t*TC:(t+1)*TC] with RoPE applied."""
                raw = rope_t.tile([P, TC], F32, name="raw")
                shuf = rope_t.tile([P, TC], F32, name="shuf")
                t1 = rope_t.tile([P, TC], F32, name="t1")
                t2 = rope_t.tile([P, TC], F32, name="t2")
                nc.vector.tensor_copy(raw[:], ps_tile[:])
                # swap 32-partition halves within each 64-row head block;
                # dep-gated DMAs get their own queue so they can't
                # head-of-line-block prefetches
                for a, b in ((0, 32), (32, 0), (64, 96), (96, 64)):
                    shuf_eng.dma_start(shuf[a : a + 32, :], raw[b : b + 32, :])
                cs = cos_sbs[t // 2][:, ts(t % 2, TC)]
                sn = sin_sbs[t // 2][:, ts(t % 2, TC)]
                nc.vector.tensor_mul(t1[:], raw[:], cs)
                if t2_gps:
                    nc.gpsimd.tensor_mul(t2[:], shuf[:], sn)
                else:
                    nc.vector.tensor_mul(t2[:], shuf[:], sn)
                nc.vector.tensor_add(dst[:, ts(t, TC)], t1[:], t2[:])

            def fetch_x(t):
                xt = xpool.tile([P, HC, TC], F32R, name="xt")
                src = xT_d[:, ts(t, TC)].rearrange("(c p) f -> p c f", p=P)
                nc.sync.dma_start(xt[:], r(src))
                return [xt[:, h, :] for h in range(HC)]

            def v_chunk(t, xts, ps_qk):
                for s in range(TC // P):  # 4 token tiles per chunk
                    tt = t * (TC // P) + s
                    vp = ps_qk.tile(
                        [P, E_LOCAL], F32, name="vp",
                        tag=("qp" if s % 2 == 0 else "kp"), bufs=1,
                    )
                    for h in range(HC):
                        nc.tensor.matmul(
                            vp[:], xts[h][:, ts(s, P)], wv_sbs[h // HH][:, h % HH, :],
                            start=(h == 0), stop=(h == HC - 1),
                        )
                    if t < 2:
                        nc.scalar.copy(
                            v4[:, tt, :, 0:HD],
                            vp[:].rearrange("p (g c) -> p g c", g=4),
                        )
                    else:
                        nc.vector.tensor_copy(
                            v4[:, tt, :, 0:HD],
                            vp[:].rearrange("p (g c) -> p g c", g=4),
                        )

            def qk_chunk(pair, t, xts, ps_qk, with_v=False):
                qp = ps_qk.tile([P, TC], F32, name="qp", tag="qp", bufs=1)
                for h in range(HC):
                    nc.tensor.matmul(
                        qp[:], wq_sbs[h // HH][:, h % HH, ts(pair, P)], xts[h][:],
                        start=(h == 0), stop=(h == HC - 1),
                    )
                rope_chunk(q_ro[pair], qp, t, nc.sync, t2_gps=(pair == 0))
                kp = ps_qk.tile([P, TC], F32, name="kp", tag="kp", bufs=1)
                for h in range(HC):
                    nc.tensor.matmul(
                        kp[:], wk_sbs[h // HH][:, h % HH, ts(pair, P)], xts[h][:],
                        start=(h == 0), stop=(h == HC - 1),
                    )
                rope_chunk(k_ro[pair], kp, t, nc.sync, t2_gps=(pair == 0))
                if with_v:
                    v_chunk(t, xts, ps_qk)

            def attn_kt(pair, c, kt, ot, ps_st):
                """One kt tile of scores+exp+PV for query chunk c."""
                st = ps_st.tile([P, 2 * TC], F32, name="st", tag="st")
                nc.tensor.matmul(
                    st[:, 0:TC],
                    k_ro[pair][0:HD, ts(kt, P)],
                    q_ro[pair][0:HD, ts(c, TC)],
                    start=True, stop=True,
                )
                nc.tensor.matmul(
                    st[:, TC : 2 * TC],
                    k_ro[pair][HD:P, ts(kt, P)],
                    q_ro[pair][HD:P, ts(c, TC)],
                    start=True, stop=True,
                    tile_position=(64, 0),
                )
                ex = expp.tile([P, 2 * TC], BF16, name="ex")
                nc.scalar.activation(ex[:], st[:], AF.Exp, scale=SCALE)
                for hd_i in range(2):
                    g = 2 * pair + hd_i
                    nc.tensor.matmul(
                        ot[:, ts(hd_i, TC)],
                        v_all[:, kt, g * (HD + 1) : (g + 1) * (HD + 1)],
                        ex[:, ts(hd_i, TC)],
                        start=(kt == 0), stop=(kt == N_TT - 1),
                    )

            def attn_finish(pair, c, ot, last=False):
                """Extract softmax sums, build 1/sum broadcast, normalize
                straight out of the ot psum into o_sb."""
                teng = nc.sync if last else nc.gpsimd
                srow = nrm.tile([1, 2 * TC], F32, name="srow")
                nc.vector.tensor_copy(srow[:], ot[HD : HD + 1, :])
                oun = nrm.tile([HD, 2 * TC], F32, name="oun")
                nc.vector.tensor_copy(oun[:], ot[0:HD, :])
                s32 = nrm.tile([32, 2 * TC // 32], F32, name="s32")
                teng.dma_start(
                    s32[:], srow[:].rearrange("p (a b) -> p a b", a=32)
                )
                nc.vector.reciprocal(s32[:], s32[:])
                invrow = nrm.tile([1, 2 * TC], F32, name="invrow")
                teng.dma_start(
                    invrow[:].rearrange("p (a b) -> p a b", a=32), s32[:]
                )
                bsum = nrm.tile([HD, 2 * TC], F32, name="bsum")
                nc.gpsimd.partition_broadcast(bsum[:], invrow[:])
                for hd_i in range(2):
                    dsts = o_sb[pair]
                    if hd_i == 0:
                        nc.vector.tensor_mul(
                            dsts[0:HD, ts(c, TC)],
                            oun[:, ts(hd_i, TC)],
                            bsum[:, ts(hd_i, TC)],
                        )
                    else:
                        onrm = nrm.tile([HD, TC], F32, name="onrm")
                        nc.vector.tensor_mul(
                            onrm[:], oun[:, ts(hd_i, TC)], bsum[:, ts(hd_i, TC)]
                        )
                        teng.dma_start(dsts[HD:P, ts(c, TC)], r(onrm[:]))

            def attention_c(pair, c, ps_st, ps_ot, last=False):
                ot = ps_ot.tile([HD + 1, 2 * TC], F32, name="ot", bufs=1)
                for kt in range(N_TT):
                    attn_kt(pair, c, kt, ot, ps_st)
                attn_finish(pair, c, ot, last=last)

            def o_proj_chunk(t, ps_qk, last=False):
                for fc in range(HC):
                    op = ps_qk.tile(
                        [P, TC], F32, name="op", tag=("qp" if fc % 2 == 0 else "kp"), bufs=1
                    )
                    for pair in range(N_PAIRS):
                        nc.tensor.matmul(
                            op[:],
                            wo_sb[:, pair, ts(fc, P)],
                            o_sb[pair][:, ts(t, TC)],
                            start=(pair == 0), stop=(pair == N_PAIRS - 1),
                        )
                    ob = outst.tile([P, TC], F32, name="ob")
                    if last and fc % 2 == 1:
                        nc.scalar.copy(ob[:], op[:])
                    else:
                        nc.vector.tensor_copy(ob[:], op[:])
                    nc.sync.dma_start(out_d[ts(fc, P), ts(t, TC)], ob[:])

            def o_proj_passA(t, ps_qk):
                obAs = []
                for fc in range(HC):
                    op = ps_qk.tile(
                        [P, TC], F32, name="op", tag=("qp" if fc % 2 == 0 else "kp"), bufs=1
                    )
                    nc.tensor.matmul(
                        op[:], wo_sb[:, 0, ts(fc, P)], o_sb[0][:, ts(t, TC)],
                        start=True, stop=True,
                    )
                    obA = oacc.tile([P, TC], F32, name="obA")
                    nc.vector.tensor_copy(obA[:], op[:])
                    obAs.append(obA)
                return obAs

            def o_proj_passB(t, obAs, ps_qk, ps_st):
                for fc in range(HC):
                    if fc % 2 == 0:
                        op = ps_qk.tile(
                            [P, TC], F32, name="op",
                            tag=("qp" if fc % 4 == 0 else "kp"), bufs=1,
                        )
                    else:
                        op = ps_st.tile([P, TC], F32, name="op2", tag="st")
                    nc.tensor.matmul(
                        op[:], wo_sb[:, 1, ts(fc, P)], o_sb[1][:, ts(t, TC)],
                        start=True, stop=True,
                    )
                    ob = outst.tile([P, TC], F32, name="ob")
                    nc.vector.tensor_add(ob[:], op[:], obAs[fc][:])
                    nc.sync.dma_start(out_d[ts(fc, P), ts(t, TC)], ob[:])

            # ---- emission order drives scheduler priority ----
            # PSUM banks: qp+kp (2) + st (2 bufs x 2) + ot (2) = 8.
            with tc.tile_pool(name="ps_qk", bufs=1, space="PSUM") as ps_qk:
                with tc.tile_pool(name="ps_st", bufs=2, space="PSUM") as ps_st:
                    with tc.tile_pool(name="ps_ot", bufs=1, space="PSUM") as ps_ot:
                        # projection t-loop for pair 0 with attention (c=0)
                        # kt-blocks interleaved so exp starts early
                        ot0 = ps_ot.tile([HD + 1, 2 * TC], F32, name="ot", bufs=1)
                        x0v = [x0ts[h][:] for h in range(HC)]
                        x1v = [xt1[:, h, :] for h in range(HC)]
                        pre = {0: x0v, 1: x1v}
                        for t in range(N_TC):
                            xts = pre[t] if t in pre else fetch_x(t)
                            qk_chunk(0, t, xts, ps_qk, with_v=True)
                            for kt in range(4 * t, 4 * t + 4):
                                attn_kt(0, 0, kt, ot0, ps_st)
                        attn_finish(0, 0, ot0)
                        for c in range(1, N_TC):
                            attention_c(0, c, ps_st, ps_ot)
                        # pair 1 projections (refetch x), overlaps attention
                        # pair 0 via scheduler priority
                        for t in range(N_TC):
                            xts = fetch_x(t)
                            qk_chunk(1, t, xts, ps_qk)
                        obAs = None
                        for c in range(N_TC):
                            attention_c(1, c, ps_st, ps_ot, last=(c == N_TC - 1))
                            if c >= 1:
                                o_proj_chunk(c - 1, ps_qk)
                            if c == 2:
                                obAs = o_proj_passA(N_TC - 1, ps_qk)
                        # keep the PE clock warm across the final
                        # normalization chain (writes a scratch psum tile
                        # nothing reads)
                        for wb in range(2):
                            warm = ps_st.tile(
                                [P, 2 * TC], F32, name="st", tag="st"
                            )
                            for w in range(12):
                                nc.tensor.matmul(
                                    warm[:, 0:TC],
                                    wo_sb[:, 0, 0:P],
                                    o_sb[0][:, ts(N_TC - 1, TC)],
                                    start=(w == 0), stop=(w == 11),
                                )
                        o_proj_passB(N_TC - 1, obAs, ps_qk, ps_st)

    nc.compile()
    return nc, names


_CACHE = {}


def _get_program():
    if "prog" not in _CACHE:
        _CACHE["prog"] = build_program()
    return _CACHE["prog"]


def _rope_tables():
    inv_freq = 1.0 / (BASE ** (np.arange(0, HD, 2, dtype=np.float64) / HD))
    t = np.arange(L, dtype=np.float64)
    freqs = np.outer(t, inv_freq)            # [L, 32]
    emb = np.concatenate((freqs, freqs), -1)  # [L, 64]
    cos = np.cos(emb).T.astype(np.float32)    # [64, L]
    sin = np.sin(emb).T.astype(np.float32)    # [64, L]
    sin_signed = sin.copy()
    sin_signed[: HD // 2] *= -1.0             # rotate_half sign baked in
    cosT = np.ascontiguousarray(np.concatenate([cos, cos], 0))      # [128, L]
    sinT = np.ascontiguousarray(np.concatenate([sin_signed, sin_signed], 0))
    return cosT, sinT


def make_in_maps(names, x, Wq, Wk, Wv, Wo):
    cosT, sinT = _rope_tables()
    in_maps = []
    xTs = [np.ascontiguousarray(x[b].T) for b in range(B)]
    for core in range(8):
        b = core // 4
        g = core % 4
        es = slice(g * E_LOCAL, (g + 1) * E_LOCAL)
        m = {
            names["in"][0]: xTs[b],
            names["in"][1]: np.ascontiguousarray(Wq[es, :].T),   # [1024, 256]
            names["in"][2]: np.ascontiguousarray(Wk[es, :].T),
            names["in"][3]: np.ascontiguousarray(Wv[es, :].T),
            names["in"][4]: np.ascontiguousarray(Wo[:, es].T),   # [256, 1024]
            names["in"][5]: cosT,
            names["in"][6]: sinT,
        }
        in_maps.append(m)
    return in_maps


def gather_out(names, res):
    out = np.zeros((B, L, HIDDEN), dtype=np.float32)
    for b in range(B):
        acc = np.zeros((HIDDEN, L), dtype=np.float32)
        for g in range(4):
            acc += res.results[b * 4 + g][names["out"]]
        out[b] = acc.T
    return out


def kernel(x, Wq, Wk, Wv, Wo):
    x = np.asarray(x, dtype=np.float32)
    Wq = np.asarray(Wq, dtype=np.float32)
    Wk = np.asarray(Wk, dtype=np.float32)
    Wv = np.asarray(Wv, dtype=np.float32)
    Wo = np.asarray(Wo, dtype=np.float32)

    nc, names = _get_program()
    in_maps = make_in_maps(names, x, Wq, Wk, Wv, Wo)
    res = run_bass_kernel_spmd(nc, in_maps, core_ids=list(range(8)))
    return gather_out(names, res)


# revision 23
# speedup vs baseline: 1.2283x; 1.2283x over previous
"""Trainium2 Bass kernel: 16-head RoPE attention (B=2, L=2048, HIDDEN=1024).

Sharding: 8 cores = 2 batches x 4 head-groups (4 heads per core).
Each core computes q/k/v projections for its 4 heads (feature-major),
RoPE, scores-transposed [k,q] per head, exp (no max subtraction --
scores are ~N(0,1)), PV with a ones-column in V to get softmax sums,
normalization, and a partial output projection [1024, 2048].
Host sums the 4 partials per batch and transposes back.

v2 schedule: chunked weight DMAs for a fast start, attention for
(pair 0, c=0) interleaved into the projection t-loop so the scalar
engine (exp) saturates early, and the softmax-sum broadcast done with
a stride-0 DMA instead of gpsimd PartitionBroadcast.
"""

import numpy as np
from contextlib import ExitStack

from concourse import bacc, tile, mybir
from concourse.bass import ts
from concourse.bass_utils import run_bass_kernel_spmd

HIDDEN = 1024
HEADS = 16
HD = 64
L = 2048
B = 2
BASE = 10000.0

P = 128
E_LOCAL = 256          # 4 heads per core
N_PAIRS = 2            # head pairs per core (2 heads on 128 partitions)
HC = HIDDEN // P       # 8 hidden chunks
TC = 512               # token chunk (matmul free dim)
N_TC = L // TC         # 4
N_TT = L // P          # 16 token tiles (for v / k-tiles)
SCALE = 1.0 / 8.0      # 1/sqrt(HD)

F32 = mybir.dt.float32
F32R = mybir.dt.float32r
BF16 = mybir.dt.bfloat16
AF = mybir.ActivationFunctionType
ALU = mybir.AluOpType


def r(ap):
    """View an fp32 AP as float32r for full-rate PE matmuls."""
    return ap.bitcast(F32R)


def build_program():
    nc = bacc.Bacc(None, target_bir_lowering=False)
    names = {}
    with tile.TileContext(nc) as tc:
        ctx = ExitStack()
        with ctx:
            dram = ctx.enter_context(tc.tile_pool(name="dram", bufs=1, space="DRAM"))
            xT_d = dram.tile([HIDDEN, L], F32, kind="ExternalInput", name="xT")
            wq_d = dram.tile([HIDDEN, E_LOCAL], F32, kind="ExternalInput", name="wq")
            wk_d = dram.tile([HIDDEN, E_LOCAL], F32, kind="ExternalInput", name="wk")
            wv_d = dram.tile([HIDDEN, E_LOCAL], F32, kind="ExternalInput", name="wv")
            wo_d = dram.tile([E_LOCAL, HIDDEN], F32, kind="ExternalInput", name="wo")
            cos_d = dram.tile([P, L], F32, kind="ExternalInput", name="cosT")
            sin_d = dram.tile([P, L], F32, kind="ExternalInput", name="sinT")
            out_d = dram.tile([HIDDEN, L], F32, kind="ExternalOutput", name="outT")
            names["in"] = [t.tensor.name for t in (xT_d, wq_d, wk_d, wv_d, wo_d, cos_d, sin_d)]
            names["out"] = out_d.tensor.name

            # ---------------- persistent SBUF ----------------
            const = ctx.enter_context(tc.tile_pool(name="const", bufs=1))
            HH = HC // 2
            wq_sbs = [const.tile([P, HH, E_LOCAL], F32R, name=f"wq{i}") for i in range(2)]
            wk_sbs = [const.tile([P, HH, E_LOCAL], F32R, name=f"wk{i}") for i in range(2)]
            wv_sbs = [const.tile([P, HH, E_LOCAL], F32R, name=f"wv{i}") for i in range(2)]
            wo_sb = const.tile([P, 2, HIDDEN], F32R)
            cos_sbs = [const.tile([P, L // 2], F32, name=f"cos{i}") for i in range(2)]
            sin_sbs = [const.tile([P, L // 2], F32, name=f"sin{i}") for i in range(2)]
            # head loads: the sync sequencer's direct DMAs occupy it
            # proportionally to bytes, so interleave half-tensors in the
            # exact order the first projection matmuls consume them
            wq_v = wq_d[:].rearrange("(c p) e -> p c e", p=P)
            wk_v = wk_d[:].rearrange("(c p) e -> p c e", p=P)
            wv_v = wv_d[:].rearrange("(c p) e -> p c e", p=P)
            # ---------------- pools ----------------
            xpool = ctx.enter_context(tc.tile_pool(name="xpool", bufs=2))
            rope_t = ctx.enter_context(tc.tile_pool(name="rope", bufs=2))
            expp = ctx.enter_context(tc.tile_pool(name="expp", bufs=5))
            nrm = ctx.enter_context(tc.tile_pool(name="nrm", bufs=1))
            outst = ctx.enter_context(tc.tile_pool(name="outst", bufs=4))
            oacc = ctx.enter_context(tc.tile_pool(name="oacc", bufs=8))
            x0ts = [oacc.tile([P, TC], F32R, name="obA") for _ in range(HC)]
            nc.sync.dma_start(wq_sbs[0][:], r(wq_v[:, 0:HH, :]))
            nc.scalar.dma_start(wk_sbs[0][:], r(wk_v[:, 0:HH, :]))
            for h in range(HH):
                nc.sync.dma_start(x0ts[h][:], r(xT_d[ts(h, P), ts(0, TC)]))
            nc.scalar.dma_start(wk_sbs[1][:], r(wk_v[:, HH:HC, :]))
            nc.sync.dma_start(wq_sbs[1][:], r(wq_v[:, HH:HC, :]))
            for h in range(HH, HC):
                nc.sync.dma_start(x0ts[h][:], r(xT_d[ts(h, P), ts(0, TC)]))
            nc.scalar.dma_start(sin_sbs[0][:], sin_d[:, 0 : L // 2])
            nc.sync.dma_start(cos_sbs[0][:], cos_d[:, 0 : L // 2])
            nc.scalar.dma_start(sin_sbs[1][:], sin_d[:, L // 2 :])
            nc.sync.dma_start(wv_sbs[0][:], r(wv_v[:, 0:HH, :]))
            nc.sync.dma_start(wv_sbs[1][:], r(wv_v[:, HH:HC, :]))
            nc.scalar.dma_start(wo_sb[:], r(wo_d[:].rearrange("(c p) f -> p c f", p=P)))
            xt1 = xpool.tile([P, HC, TC], F32R, name="xt")
            nc.sync.dma_start(
                xt1[:], r(xT_d[:, ts(1, TC)].rearrange("(c p) f -> p c f", p=P))
            )
            nc.sync.dma_start(cos_sbs[1][:], cos_d[:, L // 2 :])

            # rope'd q and k, feature-major: per pair [128, L]
            qkro = ctx.enter_context(tc.tile_pool(name="qkro", bufs=1))
            q_ro = [qkro.tile([P, L], F32R, name=f"q_ro{p}") for p in range(N_PAIRS)]
            k_ro = [qkro.tile([P, L], F32R, name=f"k_ro{p}") for p in range(N_PAIRS)]
            # v token-major with ones columns: [128 tok, tt, 4*65]
            v_all = qkro.tile([P, N_TT, 4 * (HD + 1)], BF16)
            v4 = v_all[:].rearrange("p t (g c) -> p t g c", g=4)
            ones_sb = qkro.tile([P, N_TT], BF16)
            nc.vector.memset(ones_sb[:], 1.0)
            for g in range(4):
                nc.vector.tensor_copy(
                    v_all[:, :, g * (HD + 1) + HD : g * (HD + 1) + HD + 1],
                    ones_sb[:].rearrange("p (a b) -> p a b", b=1),
                )
            # normalized attention output, feature-major per pair [128, L]
            o_sb = [qkro.tile([P, L], F32R, name=f"o_sb{p}") for p in range(N_PAIRS)]


            def rope_chunk(dst, ps_tile, t, shuf_eng, t2_gps=False):
                """psum [128, TC] -> dst[:, t*TC:(t+1)*TC] with RoPE applied."""
                raw = rope_t.tile([P, TC], F32, name="raw")
                shuf = rope_t.tile([P, TC], F32, name="shuf")
                t1 = rope_t.tile([P, TC], F32, name="t1")
                t2 = rope_t.tile([P, TC], F32, name="t2")
                nc.vector.tensor_copy(raw[:], ps_tile[:])
                # swap 32-partition halves within each 64-row head block;
                # dep-gated DMAs get their own queue so they can't
                # head-of-line-block prefetches
                for a, b in ((0, 32), (32, 0), (64, 96), (96, 64)):
                    shuf_eng.dma_start(shuf[a : a + 32, :], raw[b : b + 32, :])
                cs = cos_sbs[t // 2][:, ts(t % 2, TC)]
                sn = sin_sbs[t // 2][:, ts(t % 2, TC)]
                nc.vector.tensor_mul(t1[:], raw[:], cs)
                if t2_gps:
                    nc.gpsimd.tensor_mul(t2[:], shuf[:], sn)
                else:
                    nc.vector.tensor_mul(t2[:], shuf[:], sn)
                nc.vector.tensor_add(dst[:, ts(t, TC)], t1[:], t2[:])

            def fetch_x(t):
                xt = xpool.tile([P, HC, TC], F32R, name="xt")
                src = xT_d[:, ts(t, TC)].rearrange("(c p) f -> p c f", p=P)
                nc.sync.dma_start(xt[:], r(src))
                return [xt[:, h, :] for h in range(HC)]

            def v_chunk(t, xts, ps_qk):
                for s in range(TC // P):  # 4 token tiles per chunk
                    tt = t * (TC // P) + s
                    vp = ps_qk.tile(
                        [P, E_LOCAL], F32, name="vp",
                        tag=("qp" if s % 2 == 0 else "kp"), bufs=1,
                    )
                    for h in range(HC):
                        nc.tensor.matmul(
                            vp[:], xts[h][:, ts(s, P)], wv_sbs[h // HH][:, h % HH, :],
                            start=(h == 0), stop=(h == HC - 1),
                        )
                    if t < 2:
                        nc.scalar.copy(
                            v4[:, tt, :, 0:HD],
                            vp[:].rearrange("p (g c) -> p g c", g=4),
                        )
                    else:
                        nc.vector.tensor_copy(
                            v4[:, tt, :, 0:HD],
                            vp[:].rearrange("p (g c) -> p g c", g=4),
                        )

            def qk_chunk(pair, t, xts, ps_qk, with_v=False):
                qp = ps_qk.tile([P, TC], F32, name="qp", tag="qp", bufs=1)
                for h in range(HC):
                    nc.tensor.matmul(
                        qp[:], wq_sbs[h // HH][:, h % HH, ts(pair, P)], xts[h][:],
                        start=(h == 0), stop=(h == HC - 1),
                    )
                rope_chunk(q_ro[pair], qp, t, nc.sync, t2_gps=(pair == 0))
                kp = ps_qk.tile([P, TC], F32, name="kp", tag="kp", bufs=1)
                for h in range(HC):
                    nc.tensor.matmul(
                        kp[:], wk_sbs[h // HH][:, h % HH, ts(pair, P)], xts[h][:],
                        start=(h == 0), stop=(h == HC - 1),
                    )
                rope_chunk(k_ro[pair], kp, t, nc.sync, t2_gps=(pair == 0))
                if with_v:
                    v_chunk(t, xts, ps_qk)

            def attn_scores_exp(pair, c, kt, ps_st):
                """Scores + exp for one kt tile; returns the ex tile."""
                st = ps_st.tile([P, 2 * TC], F32, name="st", tag="st")
                nc.tensor.matmul(
                    st[:, 0:TC],
                    k_ro[pair][0:HD, ts(kt, P)],
                    q_ro[pair][0:HD, ts(c, TC)],
                    start=True, stop=True,
                )
                nc.tensor.matmul(
                    st[:, TC : 2 * TC],
                    k_ro[pair][HD:P, ts(kt, P)],
                    q_ro[pair][HD:P, ts(c, TC)],
                    start=True, stop=True,
                    tile_position=(64, 0),
                )
                ex = expp.tile([P, 2 * TC], BF16, name="ex")
                nc.scalar.activation(ex[:], st[:], AF.Exp, scale=SCALE)
                return ex

            def attn_pv(pair, kt, ot, ex):
                for hd_i in range(2):
                    g = 2 * pair + hd_i
                    nc.tensor.matmul(
                        ot[:, ts(hd_i, TC)],
                        v_all[:, kt, g * (HD + 1) : (g + 1) * (HD + 1)],
                        ex[:, ts(hd_i, TC)],
                        start=(kt == 0), stop=(kt == N_TT - 1),
                    )

            def attn_kt(pair, c, kt, ot, ps_st):
                attn_pv(pair, kt, ot, attn_scores_exp(pair, c, kt, ps_st))

            def attn_finish(pair, c, ot, last=False):
                """Extract softmax sums, build 1/sum broadcast, normalize
                straight out of the ot psum into o_sb."""
                teng = nc.sync if last else nc.gpsimd
                srow = nrm.tile([1, 2 * TC], F32, name="srow")
                nc.vector.tensor_copy(srow[:], ot[HD : HD + 1, :])
                oun = nrm.tile([HD, 2 * TC], F32, name="oun")
                nc.vector.tensor_copy(oun[:], ot[0:HD, :])
                s32 = nrm.tile([32, 2 * TC // 32], F32, name="s32")
                teng.dma_start(
                    s32[:], srow[:].rearrange("p (a b) -> p a b", a=32)
                )
                nc.vector.reciprocal(s32[:], s32[:])
                invrow = nrm.tile([1, 2 * TC], F32, name="invrow")
                teng.dma_start(
                    invrow[:].rearrange("p (a b) -> p a b", a=32), s32[:]
                )
                bsum = nrm.tile([HD, 2 * TC], F32, name="bsum")
                nc.gpsimd.partition_broadcast(bsum[:], invrow[:])
                for hd_i in range(2):
                    dsts = o_sb[pair]
                    if hd_i == 0:
                        nc.vector.tensor_mul(
                            dsts[0:HD, ts(c, TC)],
                            oun[:, ts(hd_i, TC)],
                            bsum[:, ts(hd_i, TC)],
                        )
                    else:
                        onrm = nrm.tile([HD, TC], F32, name="onrm")
                        nc.vector.tensor_mul(
                            onrm[:], oun[:, ts(hd_i, TC)], bsum[:, ts(hd_i, TC)]
                        )
                        teng.dma_start(dsts[HD:P, ts(c, TC)], r(onrm[:]))

            def attention_c(pair, c, ps_st, ps_ot, last=False):
                ot = ps_ot.tile([HD + 1, 2 * TC], F32, name="ot", bufs=1)
                exs = {kt: attn_scores_exp(pair, c, kt, ps_st) for kt in (0, 1)}
                for kt in range(N_TT):
                    if kt + 2 < N_TT:
                        exs[kt + 2] = attn_scores_exp(pair, c, kt + 2, ps_st)
                    attn_pv(pair, kt, ot, exs.pop(kt))
                attn_finish(pair, c, ot, last=last)

            def o_proj_chunk(t, ps_qk, last=False):
                for fc in range(HC):
                    op = ps_qk.tile(
                        [P, TC], F32, name="op", tag=("qp" if fc % 2 == 0 else "kp"), bufs=1
                    )
                    for pair in range(N_PAIRS):
                        nc.tensor.matmul(
                            op[:],
                            wo_sb[:, pair, ts(fc, P)],
                            o_sb[pair][:, ts(t, TC)],
                            start=(pair == 0), stop=(pair == N_PAIRS - 1),
                        )
                    ob = outst.tile([P, TC], F32, name="ob")
                    if last and fc % 2 == 1:
                        nc.scalar.copy(ob[:], op[:])
                    else:
                        nc.vector.tensor_copy(ob[:], op[:])
                    nc.sync.dma_start(out_d[ts(fc, P), ts(t, TC)], ob[:])

            def o_proj_passA(t, ps_qk):
                obAs = []
                for fc in range(HC):
                    op = ps_qk.tile(
                        [P, TC], F32, name="op", tag=("qp" if fc % 2 == 0 else "kp"), bufs=1
                    )
                    nc.tensor.matmul(
                        op[:], wo_sb[:, 0, ts(fc, P)], o_sb[0][:, ts(t, TC)],
                        start=True, stop=True,
                    )
                    obA = oacc.tile([P, TC], F32, name="obA")
                    nc.vector.tensor_copy(obA[:], op[:])
                    obAs.append(obA)
                return obAs

            def o_proj_passB(t, obAs, ps_qk, ps_st):
                for fc in range(HC):
                    if fc % 2 == 0:
                        op = ps_qk.tile(
                            [P, TC], F32, name="op",
                            tag=("qp" if fc % 4 == 0 else "kp"), bufs=1,
                        )
                    else:
                        op = ps_st.tile([P, TC], F32, name="op2", tag="st")
                    nc.tensor.matmul(
                        op[:], wo_sb[:, 1, ts(fc, P)], o_sb[1][:, ts(t, TC)],
                        start=True, stop=True,
                    )
                    ob = outst.tile([P, TC], F32, name="ob")
                    nc.vector.tensor_add(ob[:], op[:], obAs[fc][:])
                    nc.sync.dma_start(out_d[ts(fc, P), ts(t, TC)], ob[:])

            # ---- emission order drives scheduler priority ----
            # PSUM banks: qp+kp (2) + st (2 bufs x 2) + ot (2) = 8.
            with tc.tile_pool(name="ps_qk", bufs=1, space="PSUM") as ps_qk:
                with tc.tile_pool(name="ps_st", bufs=2, space="PSUM") as ps_st:
                    with tc.tile_pool(name="ps_ot", bufs=1, space="PSUM") as ps_ot:
                        # projection t-loop for pair 0 with attention (c=0)
                        # kt-blocks interleaved so exp starts early
                        ot0 = ps_ot.tile([HD + 1, 2 * TC], F32, name="ot", bufs=1)
                        x0v = [x0ts[h][:] for h in range(HC)]
                        x1v = [xt1[:, h, :] for h in range(HC)]
                        pre = {0: x0v, 1: x1v}
                        for t in range(N_TC):
                            xts = pre[t] if t in pre else fetch_x(t)
                            qk_chunk(0, t, xts, ps_qk, with_v=True)
                            for kt in range(4 * t, 4 * t + 4):
                                attn_kt(0, 0, kt, ot0, ps_st)
                        attn_finish(0, 0, ot0)
                        for c in range(1, N_TC):
                            attention_c(0, c, ps_st, ps_ot)
                        # pair 1 projections (refetch x), overlaps attention
                        # pair 0 via scheduler priority
                        for t in range(N_TC):
                            xts = fetch_x(t)
                            qk_chunk(1, t, xts, ps_qk)
                        obAs = None
                        for c in range(N_TC):
                            attention_c(1, c, ps_st, ps_ot, last=(c == N_TC - 1))
                            if c >= 1:
                                o_proj_chunk(c - 1, ps_qk)
                            if c == 2:
                                obAs = o_proj_passA(N_TC - 1, ps_qk)
                        # keep the PE clock warm across the final
                        # normalization chain (writes a scratch psum tile
                        # nothing reads)
                        for wb in range(2):
                            warm = ps_st.tile(
                                [P, 2 * TC], F32, name="st", tag="st"
                            )
                            for w in range(12):
                                nc.tensor.matmul(
                                    warm[:, 0:TC],
                                    wo_sb[:, 0, 0:P],
                                    o_sb[0][:, ts(N_TC - 1, TC)],
                                    start=(w == 0), stop=(w == 11),
                                )
                        o_proj_passB(N_TC - 1, obAs, ps_qk, ps_st)

    nc.compile()
    return nc, names


_CACHE = {}


def _get_program():
    if "prog" not in _CACHE:
        _CACHE["prog"] = build_program()
    return _CACHE["prog"]


def _rope_tables():
    inv_freq = 1.0 / (BASE ** (np.arange(0, HD, 2, dtype=np.float64) / HD))
    t = np.arange(L, dtype=np.float64)
    freqs = np.outer(t, inv_freq)            # [L, 32]
    emb = np.concatenate((freqs, freqs), -1)  # [L, 64]
    cos = np.cos(emb).T.astype(np.float32)    # [64, L]
    sin = np.sin(emb).T.astype(np.float32)    # [64, L]
    sin_signed = sin.copy()
    sin_signed[: HD // 2] *= -1.0             # rotate_half sign baked in
    cosT = np.ascontiguousarray(np.concatenate([cos, cos], 0))      # [128, L]
    sinT = np.ascontiguousarray(np.concatenate([sin_signed, sin_signed], 0))
    return cosT, sinT


def make_in_maps(names, x, Wq, Wk, Wv, Wo):
    cosT, sinT = _rope_tables()
    in_maps = []
    xTs = [np.ascontiguousarray(x[b].T) for b in range(B)]
    for core in range(8):
        b = core // 4
        g = core % 4
        es = slice(g * E_LOCAL, (g + 1) * E_LOCAL)
        m = {
            names["in"][0]: xTs[b],
            names["in"][1]: np.ascontiguousarray(Wq[es, :].T),   # [1024, 256]
            names["in"][2]: np.ascontiguousarray(Wk[es, :].T),
            names["in"][3]: np.ascontiguousarray(Wv[es, :].T),
            names["in"][4]: np.ascontiguousarray(Wo[:, es].T),   # [256, 1024]
            names["in"][5]: cosT,
            names["in"][6]: sinT,
        }
        in_maps.append(m)
    return in_maps


def gather_out(names, res):
    out = np.zeros((B, L, HIDDEN), dtype=np.float32)
    for b in range(B):
        acc = np.zeros((HIDDEN, L), dtype=np.float32)
        for g in range(4):
            acc += res.results[b * 4 + g][names["out"]]
        out[b] = acc.T
    return out


def kernel(x, Wq, Wk, Wv, Wo):
    x = np.asarray(x, dtype=np.float32)
    Wq = np.asarray(Wq, dtype=np.float32)
    Wk = np.asarray(Wk, dtype=np.float32)
    Wv = np.asarray(Wv, dtype=np.float32)
    Wo = np.asarray(Wo, dtype=np.float32)

    nc, names = _get_program()
    in_maps = make_in_maps(names, x, Wq, Wk, Wv, Wo)
    res = run_bass_kernel_spmd(nc, in_maps, core_ids=list(range(8)))
    return gather_out(names, res)


# revision 25
# speedup vs baseline: 1.2314x; 1.0025x over previous
"""Trainium2 Bass kernel: 16-head RoPE attention (B=2, L=2048, HIDDEN=1024).

Sharding: 8 cores = 2 batches x 4 head-groups (4 heads per core).
Each core computes q/k/v projections for its 4 heads (feature-major),
RoPE, scores-transposed [k,q] per head, exp (no max subtraction --
scores are ~N(0,1)), PV with a ones-column in V to get softmax sums,
normalization, and a partial output projection [1024, 2048].
Host sums the 4 partials per batch and transposes back.

v2 schedule: chunked weight DMAs for a fast start, attention for
(pair 0, c=0) interleaved into the projection t-loop so the scalar
engine (exp) saturates early, and the softmax-sum broadcast done with
a stride-0 DMA instead of gpsimd PartitionBroadcast.
"""

import numpy as np
from contextlib import ExitStack

from concourse import bacc, tile, mybir
from concourse.bass import ts
from concourse.bass_utils import run_bass_kernel_spmd

HIDDEN = 1024
HEADS = 16
HD = 64
L = 2048
B = 2
BASE = 10000.0

P = 128
E_LOCAL = 256          # 4 heads per core
N_PAIRS = 2            # head pairs per core (2 heads on 128 partitions)
HC = HIDDEN // P       # 8 hidden chunks
TC = 512               # token chunk (matmul free dim)
N_TC = L // TC         # 4
N_TT = L // P          # 16 token tiles (for v / k-tiles)
SCALE = 1.0 / 8.0      # 1/sqrt(HD)

F32 = mybir.dt.float32
F32R = mybir.dt.float32r
BF16 = mybir.dt.bfloat16
AF = mybir.ActivationFunctionType
ALU = mybir.AluOpType


def r(ap):
    """View an fp32 AP as float32r for full-rate PE matmuls."""
    return ap.bitcast(F32R)


def build_program():
    nc = bacc.Bacc(None, target_bir_lowering=False)
    names = {}
    with tile.TileContext(nc) as tc:
        ctx = ExitStack()
        with ctx:
            dram = ctx.enter_context(tc.tile_pool(name="dram", bufs=1, space="DRAM"))
            xT_d = dram.tile([HIDDEN, L], F32, kind="ExternalInput", name="xT")
            wq_d = dram.tile([HIDDEN, E_LOCAL], F32, kind="ExternalInput", name="wq")
            wk_d = dram.tile([HIDDEN, E_LOCAL], F32, kind="ExternalInput", name="wk")
            wv_d = dram.tile([HIDDEN, E_LOCAL], F32, kind="ExternalInput", name="wv")
            wo_d = dram.tile([E_LOCAL, HIDDEN], F32, kind="ExternalInput", name="wo")
            cos_d = dram.tile([P, L], F32, kind="ExternalInput", name="cosT")
            sin_d = dram.tile([P, L], F32, kind="ExternalInput", name="sinT")
            out_d = dram.tile([HIDDEN, L], F32, kind="ExternalOutput", name="outT")
            names["in"] = [t.tensor.name for t in (xT_d, wq_d, wk_d, wv_d, wo_d, cos_d, sin_d)]
            names["out"] = out_d.tensor.name

            # ---------------- persistent SBUF ----------------
            const = ctx.enter_context(tc.tile_pool(name="const", bufs=1))
            HH = HC // 2
            wq_sbs = [const.tile([P, HH, E_LOCAL], F32R, name=f"wq{i}") for i in range(2)]
            wk_sbs = [const.tile([P, HH, E_LOCAL], F32R, name=f"wk{i}") for i in range(2)]
            wv_sbs = [const.tile([P, HH, E_LOCAL], F32R, name=f"wv{i}") for i in range(2)]
            wo_sb = const.tile([P, 2, HIDDEN], F32R)
            cos_sbs = [const.tile([P, L // 2], F32, name=f"cos{i}") for i in range(2)]
            sin_sbs = [const.tile([P, L // 2], F32, name=f"sin{i}") for i in range(2)]
            # head loads: the sync sequencer's direct DMAs occupy it
            # proportionally to bytes, so interleave half-tensors in the
            # exact order the first projection matmuls consume them
            wq_v = wq_d[:].rearrange("(c p) e -> p c e", p=P)
            wk_v = wk_d[:].rearrange("(c p) e -> p c e", p=P)
            wv_v = wv_d[:].rearrange("(c p) e -> p c e", p=P)
            # ---------------- pools ----------------
            xpool = ctx.enter_context(tc.tile_pool(name="xpool", bufs=2))
            rope_t = ctx.enter_context(tc.tile_pool(name="rope", bufs=2))
            expp = ctx.enter_context(tc.tile_pool(name="expp", bufs=5))
            nrm = ctx.enter_context(tc.tile_pool(name="nrm", bufs=1))
            outst = ctx.enter_context(tc.tile_pool(name="outst", bufs=4))
            oacc = ctx.enter_context(tc.tile_pool(name="oacc", bufs=8))
            x0ts = [oacc.tile([P, TC], F32R, name="obA") for _ in range(HC)]
            nc.sync.dma_start(wq_sbs[0][:], r(wq_v[:, 0:HH, :]))
            nc.scalar.dma_start(wk_sbs[0][:], r(wk_v[:, 0:HH, :]))
            for h in range(HH):
                nc.sync.dma_start(x0ts[h][:], r(xT_d[ts(h, P), ts(0, TC)]))
            nc.scalar.dma_start(wk_sbs[1][:], r(wk_v[:, HH:HC, :]))
            nc.sync.dma_start(wq_sbs[1][:], r(wq_v[:, HH:HC, :]))
            for h in range(HH, HC):
                nc.sync.dma_start(x0ts[h][:], r(xT_d[ts(h, P), ts(0, TC)]))
            nc.scalar.dma_start(sin_sbs[0][:], sin_d[:, 0 : L // 2])
            nc.sync.dma_start(cos_sbs[0][:], cos_d[:, 0 : L // 2])
            nc.scalar.dma_start(sin_sbs[1][:], sin_d[:, L // 2 :])
            nc.sync.dma_start(wv_sbs[0][:], r(wv_v[:, 0:HH, :]))
            nc.sync.dma_start(wv_sbs[1][:], r(wv_v[:, HH:HC, :]))
            nc.scalar.dma_start(wo_sb[:], r(wo_d[:].rearrange("(c p) f -> p c f", p=P)))
            xt1 = xpool.tile([P, HC, TC], F32R, name="xt")
            nc.sync.dma_start(
                xt1[:], r(xT_d[:, ts(1, TC)].rearrange("(c p) f -> p c f", p=P))
            )
            nc.sync.dma_start(cos_sbs[1][:], cos_d[:, L // 2 :])

            # rope'd q and k, feature-major: per pair [128, L]
            qkro = ctx.enter_context(tc.tile_pool(name="qkro", bufs=1))
            q_ro = [qkro.tile([P, L], F32R, name=f"q_ro{p}") for p in range(N_PAIRS)]
            k_ro = [qkro.tile([P, L], F32R, name=f"k_ro{p}") for p in range(N_PAIRS)]
            # v token-major with ones columns: [128 tok, tt, 4*65]
            v_all = qkro.tile([P, N_TT, 4 * (HD + 1)], BF16)
            v4 = v_all[:].rearrange("p t (g c) -> p t g c", g=4)
            ones_sb = qkro.tile([P, N_TT], BF16)
            nc.vector.memset(ones_sb[:], 1.0)
            for g in range(4):
                nc.vector.tensor_copy(
                    v_all[:, :, g * (HD + 1) + HD : g * (HD + 1) + HD + 1],
                    ones_sb[:].rearrange("p (a b) -> p a b", b=1),
                )
            # normalized attention output, feature-major per pair [128, L]
            o_sb = [qkro.tile([P, L], F32R, name=f"o_sb{p}") for p in range(N_PAIRS)]


            def rope_chunk(dst, ps_tile, t, shuf_eng, t2_gps=False):
                """psum [128, TC] -> dst[:, t*TC:(t+1)*TC] with RoPE applied."""
                raw = rope_t.tile([P, TC], F32, name="raw")
                shuf = rope_t.tile([P, TC], F32, name="shuf")
                t1 = rope_t.tile([P, TC], F32, name="t1")
                t2 = rope_t.tile([P, TC], F32, name="t2")
                nc.vector.tensor_copy(raw[:], ps_tile[:])
                # swap 32-partition halves within each 64-row head block;
                # dep-gated DMAs get their own queue so they can't
                # head-of-line-block prefetches
                for a, b in ((0, 32), (32, 0), (64, 96), (96, 64)):
                    shuf_eng.dma_start(shuf[a : a + 32, :], raw[b : b + 32, :])
                cs = cos_sbs[t // 2][:, ts(t % 2, TC)]
                sn = sin_sbs[t // 2][:, ts(t % 2, TC)]
                nc.vector.tensor_mul(t1[:], raw[:], cs)
                if t2_gps:
                    nc.gpsimd.tensor_mul(t2[:], shuf[:], sn)
                else:
                    nc.vector.tensor_mul(t2[:], shuf[:], sn)
                nc.vector.tensor_add(dst[:, ts(t, TC)], t1[:], t2[:])

            def fetch_x(t):
                xt = xpool.tile([P, HC, TC], F32R, name="xt")
                src = xT_d[:, ts(t, TC)].rearrange("(c p) f -> p c f", p=P)
                nc.sync.dma_start(xt[:], r(src))
                return [xt[:, h, :] for h in range(HC)]

            def v_chunk(t, xts, ps_qk):
                for s in range(TC // P):  # 4 token tiles per chunk
                    tt = t * (TC // P) + s
                    vp = ps_qk.tile(
                        [P, E_LOCAL], F32, name="vp",
                        tag=("qp" if s % 2 == 0 else "kp"), bufs=1,
                    )
                    for h in range(HC):
                        nc.tensor.matmul(
                            vp[:], xts[h][:, ts(s, P)], wv_sbs[h // HH][:, h % HH, :],
                            start=(h == 0), stop=(h == HC - 1),
                        )
                    if t < 2:
                        nc.scalar.copy(
                            v4[:, tt, :, 0:HD],
                            vp[:].rearrange("p (g c) -> p g c", g=4),
                        )
                    else:
                        nc.vector.tensor_copy(
                            v4[:, tt, :, 0:HD],
                            vp[:].rearrange("p (g c) -> p g c", g=4),
                        )

            def qk_chunk(pair, t, xts, ps_qk, with_v=False):
                qp = ps_qk.tile([P, TC], F32, name="qp", tag="qp", bufs=1)
                for h in range(HC):
                    nc.tensor.matmul(
                        qp[:], wq_sbs[h // HH][:, h % HH, ts(pair, P)], xts[h][:],
                        start=(h == 0), stop=(h == HC - 1),
                    )
                rope_chunk(q_ro[pair], qp, t, nc.sync, t2_gps=(pair == 0))
                kp = ps_qk.tile([P, TC], F32, name="kp", tag="kp", bufs=1)
                for h in range(HC):
                    nc.tensor.matmul(
                        kp[:], wk_sbs[h // HH][:, h % HH, ts(pair, P)], xts[h][:],
                        start=(h == 0), stop=(h == HC - 1),
                    )
                rope_chunk(k_ro[pair], kp, t, nc.sync, t2_gps=(pair == 0))
                if with_v:
                    v_chunk(t, xts, ps_qk)

            def attn_scores_exp(pair, c, kt, ps_st):
                """Scores + exp for one kt tile; returns the ex tile."""
                st = ps_st.tile([P, 2 * TC], F32, name="st", tag="st")
                nc.tensor.matmul(
                    st[:, 0:TC],
                    k_ro[pair][0:HD, ts(kt, P)],
                    q_ro[pair][0:HD, ts(c, TC)],
                    start=True, stop=True,
                )
                nc.tensor.matmul(
                    st[:, TC : 2 * TC],
                    k_ro[pair][HD:P, ts(kt, P)],
                    q_ro[pair][HD:P, ts(c, TC)],
                    start=True, stop=True,
                    tile_position=(64, 0),
                )
                ex = expp.tile([P, 2 * TC], BF16, name="ex")
                nc.scalar.activation(ex[:], st[:], AF.Exp, scale=SCALE)
                return ex

            def attn_pv(pair, kt, ot, ex):
                for hd_i in range(2):
                    g = 2 * pair + hd_i
                    nc.tensor.matmul(
                        ot[:, ts(hd_i, TC)],
                        v_all[:, kt, g * (HD + 1) : (g + 1) * (HD + 1)],
                        ex[:, ts(hd_i, TC)],
                        start=(kt == 0), stop=(kt == N_TT - 1),
                    )

            def attn_kt(pair, c, kt, ot, ps_st):
                attn_pv(pair, kt, ot, attn_scores_exp(pair, c, kt, ps_st))

            def attn_finish(pair, c, ot, last=False):
                """Extract softmax sums, build 1/sum broadcast, normalize
                straight out of the ot psum into o_sb."""
                teng = nc.sync if last else nc.gpsimd
                srow = nrm.tile([1, 2 * TC], F32, name="srow")
                if last:
                    nc.scalar.copy(srow[:], ot[HD : HD + 1, :])
                else:
                    nc.vector.tensor_copy(srow[:], ot[HD : HD + 1, :])
                oun = nrm.tile([HD, 2 * TC], F32, name="oun")
                nc.vector.tensor_copy(oun[:], ot[0:HD, :])
                s32 = nrm.tile([32, 2 * TC // 32], F32, name="s32")
                teng.dma_start(
                    s32[:], srow[:].rearrange("p (a b) -> p a b", a=32)
                )
                nc.vector.reciprocal(s32[:], s32[:])
                invrow = nrm.tile([1, 2 * TC], F32, name="invrow")
                teng.dma_start(
                    invrow[:].rearrange("p (a b) -> p a b", a=32), s32[:]
                )
                bsum = nrm.tile([HD, 2 * TC], F32, name="bsum")
                nc.gpsimd.partition_broadcast(bsum[:], invrow[:])
                for hd_i in range(2):
                    dsts = o_sb[pair]
                    if hd_i == 0:
                        nc.vector.tensor_mul(
                            dsts[0:HD, ts(c, TC)],
                            oun[:, ts(hd_i, TC)],
                            bsum[:, ts(hd_i, TC)],
                        )
                    else:
                        onrm = nrm.tile([HD, TC], F32, name="onrm")
                        nc.vector.tensor_mul(
                            onrm[:], oun[:, ts(hd_i, TC)], bsum[:, ts(hd_i, TC)]
                        )
                        teng.dma_start(dsts[HD:P, ts(c, TC)], r(onrm[:]))

            def attention_c(pair, c, ps_st, ps_ot, last=False):
                ot = ps_ot.tile([HD + 1, 2 * TC], F32, name="ot", bufs=1)
                exs = {kt: attn_scores_exp(pair, c, kt, ps_st) for kt in (0, 1)}
                for kt in range(N_TT):
                    if kt + 2 < N_TT:
                        exs[kt + 2] = attn_scores_exp(pair, c, kt + 2, ps_st)
                    attn_pv(pair, kt, ot, exs.pop(kt))
                attn_finish(pair, c, ot, last=last)

            def o_proj_chunk(t, ps_qk, last=False):
                for fc in range(HC):
                    op = ps_qk.tile(
                        [P, TC], F32, name="op", tag=("qp" if fc % 2 == 0 else "kp"), bufs=1
                    )
                    for pair in range(N_PAIRS):
                        nc.tensor.matmul(
                            op[:],
                            wo_sb[:, pair, ts(fc, P)],
                            o_sb[pair][:, ts(t, TC)],
                            start=(pair == 0), stop=(pair == N_PAIRS - 1),
                        )
                    ob = outst.tile([P, TC], F32, name="ob")
                    if last and fc % 2 == 1:
                        nc.scalar.copy(ob[:], op[:])
                    else:
                        nc.vector.tensor_copy(ob[:], op[:])
                    nc.sync.dma_start(out_d[ts(fc, P), ts(t, TC)], ob[:])

            def o_proj_passA(t, ps_qk):
                obAs = []
                for fc in range(HC):
                    op = ps_qk.tile(
                        [P, TC], F32, name="op", tag=("qp" if fc % 2 == 0 else "kp"), bufs=1
                    )
                    nc.tensor.matmul(
                        op[:], wo_sb[:, 0, ts(fc, P)], o_sb[0][:, ts(t, TC)],
                        start=True, stop=True,
                    )
                    obA = oacc.tile([P, TC], F32, name="obA")
                    nc.vector.tensor_copy(obA[:], op[:])
                    obAs.append(obA)
                return obAs

            def o_proj_passB(t, obAs, ps_qk, ps_st):
                for fc in range(HC):
                    if fc % 2 == 0:
                        op = ps_qk.tile(
                            [P, TC], F32, name="op",
                            tag=("qp" if fc % 4 == 0 else "kp"), bufs=1,
                        )
                    else:
                        op = ps_st.tile([P, TC], F32, name="op2", tag="st")
                    nc.tensor.matmul(
                        op[:], wo_sb[:, 1, ts(fc, P)], o_sb[1][:, ts(t, TC)],
                        start=True, stop=True,
                    )
                    ob = outst.tile([P, TC], F32, name="ob")
                    nc.vector.tensor_add(ob[:], op[:], obAs[fc][:])
                    nc.sync.dma_start(out_d[ts(fc, P), ts(t, TC)], ob[:])

            # ---- emission order drives scheduler priority ----
            # PSUM banks: qp+kp (2) + st (2 bufs x 2) + ot (2) = 8.
            with tc.tile_pool(name="ps_qk", bufs=1, space="PSUM") as ps_qk:
                with tc.tile_pool(name="ps_st", bufs=2, space="PSUM") as ps_st:
                    with tc.tile_pool(name="ps_ot", bufs=1, space="PSUM") as ps_ot:
                        # projection t-loop for pair 0 with attention (c=0)
                        # kt-blocks interleaved so exp starts early
                        ot0 = ps_ot.tile([HD + 1, 2 * TC], F32, name="ot", bufs=1)
                        x0v = [x0ts[h][:] for h in range(HC)]
                        x1v = [xt1[:, h, :] for h in range(HC)]
                        pre = {0: x0v, 1: x1v}
                        for t in range(N_TC):
                            xts = pre[t] if t in pre else fetch_x(t)
                            qk_chunk(0, t, xts, ps_qk, with_v=True)
                            for kt in range(4 * t, 4 * t + 4):
                                attn_kt(0, 0, kt, ot0, ps_st)
                        attn_finish(0, 0, ot0)
                        for c in range(1, N_TC):
                            attention_c(0, c, ps_st, ps_ot)
                        # pair 1 projections (refetch x), overlaps attention
                        # pair 0 via scheduler priority
                        for t in range(N_TC):
                            xts = fetch_x(t)
                            qk_chunk(1, t, xts, ps_qk)
                        obAs = None
                        for c in range(N_TC):
                            attention_c(1, c, ps_st, ps_ot, last=(c == N_TC - 1))
                            if c >= 1:
                                o_proj_chunk(c - 1, ps_qk)
                            if c == 2:
                                obAs = o_proj_passA(N_TC - 1, ps_qk)
                        # keep the PE clock warm across the final
                        # normalization chain (writes a scratch psum tile
                        # nothing reads)
                        for wb in range(4):
                            warm = ps_st.tile(
                                [P, 2 * TC], F32, name="st", tag="st"
                            )
                            for w in range(12):
                                nc.tensor.matmul(
                                    warm[:, 0:TC],
                                    wo_sb[:, 0, 0:P],
                                    o_sb[0][:, ts(N_TC - 1, TC)],
                                    start=(w == 0), stop=(w == 11),
                                )
                        o_proj_passB(N_TC - 1, obAs, ps_qk, ps_st)

    nc.compile()
    return nc, names


_CACHE = {}


def _get_program():
    if "prog" not in _CACHE:
        _CACHE["prog"] = build_program()
    return _CACHE["prog"]


def _rope_tables():
    inv_freq = 1.0 / (BASE ** (np.arange(0, HD, 2, dtype=np.float64) / HD))
    t = np.arange(L, dtype=np.float64)
    freqs = np.outer(t, inv_freq)            # [L, 32]
    emb = np.concatenate((freqs, freqs), -1)  # [L, 64]
    cos = np.cos(emb).T.astype(np.float32)    # [64, L]
    sin = np.sin(emb).T.astype(np.float32)    # [64, L]
    sin_signed = sin.copy()
    sin_signed[: HD // 2] *= -1.0             # rotate_half sign baked in
    cosT = np.ascontiguousarray(np.concatenate([cos, cos], 0))      # [128, L]
    sinT = np.ascontiguousarray(np.concatenate([sin_signed, sin_signed], 0))
    return cosT, sinT


def make_in_maps(names, x, Wq, Wk, Wv, Wo):
    cosT, sinT = _rope_tables()
    in_maps = []
    xTs = [np.ascontiguousarray(x[b].T) for b in range(B)]
    for core in range(8):
        b = core // 4
        g = core % 4
        es = slice(g * E_LOCAL, (g + 1) * E_LOCAL)
        m = {
            names["in"][0]: xTs[b],
            names["in"][1]: np.ascontiguousarray(Wq[es, :].T),   # [1024, 256]
            names["in"][2]: np.ascontiguousarray(Wk[es, :].T),
            names["in"][3]: np.ascontiguousarray(Wv[es, :].T),
            names["in"][4]: np.ascontiguousarray(Wo[:, es].T),   # [256, 1024]
            names["in"][5]: cosT,
            names["in"][6]: sinT,
        }
        in_maps.append(m)
    return in_maps


def gather_out(names, res):
    out = np.zeros((B, L, HIDDEN), dtype=np.float32)
    for b in range(B):
        acc = np.zeros((HIDDEN, L), dtype=np.float32)
        for g in range(4):
            acc += res.results[b * 4 + g][names["out"]]
        out[b] = acc.T
    return out


def kernel(x, Wq, Wk, Wv, Wo):
    x = np.asarray(x, dtype=np.float32)
    Wq = np.asarray(Wq, dtype=np.float32)
    Wk = np.asarray(Wk, dtype=np.float32)
    Wv = np.asarray(Wv, dtype=np.float32)
    Wo = np.asarray(Wo, dtype=np.float32)

    nc, names = _get_program()
    in_maps = make_in_maps(names, x, Wq, Wk, Wv, Wo)
    res = run_bass_kernel_spmd(nc, in_maps, core_ids=list(range(8)))
    return gather_out(names, res)
